# revision 1
# baseline (speedup 1.0000x reference)
"""Trainium2 Bass kernel for nn_Conv2d_61881888800824.

StyleGAN2-style synthesis layer:
    renorm(weight) -> upsample x2 (zero-insert) -> 4x4 FIR -> 3x3 conv
    -> + noise*strength -> + bias -> leaky_relu(0.2) * sqrt(2)

Math: the upsample+FIR+conv chain is folded (host-side) into a polyphase
decomposition — 4 independent 3x3 convolutions of the ORIGINAL 64x64 input
(one per output-pixel parity phase (alpha,beta)), each expressed as
channel-contraction matmuls on the TensorEngine in float32r (tf32-like)
precision with fp32 PSUM accumulation. The sqrt(2) lrelu gain is folded
into weights/bias/noise (lrelu is positively homogeneous).

Sharding: data-parallel over batch — 1 image per NeuronCore, 8 cores.

Self-contained: inputs are the full tensors from setup_inputs(); output is
the full [8, 256, 128, 128] fp32 array.
"""
from contextlib import ExitStack

import numpy as np

import bass_rust
import concourse.bass as bass
import concourse.mybir as mybir
import concourse.tile as tile
from concourse.bass_utils import run_bass_kernel_spmd

F32R = mybir.dt.float32r
F32 = mybir.dt.float32


# ---------------------------------------------------------------------------
# Wait legalization: this walrus build accepts at most ONE embedded sync wait
# per instruction. Tile can emit more (incl. same-engine self-waits that are
# provably satisfied by the engine's serial program order). Drop the provable
# ones; move the rest onto standalone EventSemaphore instructions inserted
# just before the over-limit instruction on the same engine.
# ---------------------------------------------------------------------------

def _is_async_update(inst) -> bool:
    n = type(inst).__name__
    return 'DMA' in n or 'Swdge' in n or 'Collective' in n or 'Dma' in n


def legalize_waits(nc, evsem_limit: int = 1) -> int:
    n_fixed = 0
    for fn in nc.m.functions:
        for bb in fn.blocks:
            insts = bb.instructions
            cum: dict[tuple, int] = {}
            out = []
            changed = False
            for inst in insts:
                si = inst.sync_info
                waits = list(si.on_wait) if si is not None and si.on_wait else []
                updates = list(si.on_update) if si is not None and si.on_update else []
                eng = inst.engine
                limit = 1
                if len(waits) > limit:
                    kept = []
                    for w in waits:
                        if (w.sync_type == 'semaphore'
                                and w.wait_mode == 'sem-ge-imm'
                                and w.wait_reg is None
                                and cum.get((eng, w.id), 0) >= w.wait_value):
                            continue
                        kept.append(w)
                    waits = kept
                if len(waits) > limit:
                    excess = waits[:-limit]
                    waits = waits[-limit:]
                    while excess:
                        take, excess = excess[:evsem_limit], excess[evsem_limit:]
                        ev = mybir.InstEventSemaphore(
                            name=nc.get_next_instruction_name(), ins=[], outs=[])
                        ev.engine = eng
                        ev.sync_info = bass_rust.SyncInfo(on_wait=take, on_update=[])
                        out.append(ev)
                    inst.sync_info = bass_rust.SyncInfo(on_wait=waits,
                                                        on_update=updates)
                    changed = True
                    n_fixed += 1
                elif si is not None and len(list(si.on_wait or [])) != len(waits):
                    inst.sync_info = bass_rust.SyncInfo(on_wait=waits,
                                                        on_update=updates)
                    changed = True
                    n_fixed += 1
                out.append(inst)
                if not _is_async_update(inst):
                    for u in updates:
                        if (u.sync_type == 'semaphore'
                                and u.update_mode == 'sem-inc'
                                and u.update_reg is None):
                            k = (eng, u.id)
                            cum[k] = cum.get(k, 0) + u.update_value
            if changed:
                bb.instructions = out
    return n_fixed


# ---------------------------------------------------------------------------
# Device kernel (per core: one batch image)
# ---------------------------------------------------------------------------

def build_conv_nc(mm_dtype=F32R):
    nc = bass.Bass("TRN2", target_bir_lowering=False, debug=False)
    xin = nc.dram_tensor("xin", [512, 64, 64], mm_dtype, kind="ExternalInput").ap()
    wq = nc.dram_tensor("wq", [2, 2, 128, 72 * 128], mm_dtype,
                        kind="ExternalInput").ap()
    noise4 = nc.dram_tensor("noise4", [2, 2, 64, 64], F32, kind="ExternalInput").ap()
    biasq = nc.dram_tensor("biasq", [128, 2], F32, kind="ExternalInput").ap()
    y = nc.dram_tensor("y", [256, 128, 128], F32, kind="ExternalOutput").ap()

    with ExitStack() as ctx:
        tc = ctx.enter_context(tile.TileContext(nc))
        xp = ctx.enter_context(tc.tile_pool(name="x", bufs=1))
        wp = ctx.enter_context(tc.tile_pool(name="w", bufs=2))
        np_ = ctx.enter_context(tc.tile_pool(name="noise", bufs=1))
        bp = ctx.enter_context(tc.tile_pool(name="bias", bufs=1))
        op = ctx.enter_context(tc.tile_pool(name="out", bufs=3))
        pp = ctx.enter_context(tc.tile_pool(name="psum", bufs=8, space="PSUM"))

        bias_sb = bp.tile([128, 2], F32)
        nc.sync.dma_start(bias_sb[:], biasq)

        # hoist the first weight slab ahead of the x loads so the SDMA
        # round-robin finishes the first-matmul critical path sooner
        wslab0 = wp.tile([128, 72, 128], mm_dtype, tag="wslab")
        wsrc0 = wq[0, 0].rearrange("ci (j co) -> ci j co", co=128)
        nc.sync.dma_start(wslab0[:, 0:36, :], wsrc0[:, 0:36, :])
        nc.sync.dma_start(wslab0[:, 36:72, :], wsrc0[:, 36:72, :])

        # x with 1-pixel zero border: [128, cg, 66, 66]; row-half DMAs give
        # finer dependency granularity for the first accumulation groups
        xq = xp.tile([128, 4, 66, 66], mm_dtype)
        for cg in range(4):
            for sl in (xq[:, cg, 0, :], xq[:, cg, 65, :],
                       xq[:, cg, :, 0], xq[:, cg, :, 65]):
                nc.vector.memset(sl.bitcast(F32), 0.0)
        for cg in range(4):
            nc.sync.dma_start(xq[:, cg, 1:33, 1:65],
                              xin[cg * 128:(cg + 1) * 128, 0:32])
            nc.sync.dma_start(xq[:, cg, 33:65, 1:65],
                              xin[cg * 128:(cg + 1) * 128, 32:64])

        for a in range(2):          # output row parity (alpha)
            noise_sb = np_.tile([128, 2, 64, 64], F32)
            nsrc = bass.AP(
                tensor=noise4.tensor,
                offset=a * 2 * 4096,
                ap=[[0, 128], [4096, 2], [64, 64], [1, 64]],
            )
            nc.sync.dma_start(noise_sb[:], nsrc)
            for ct in range(2):     # cout tile of 128
                if a == 0 and ct == 0:
                    wslab = wslab0
                else:
                    wslab = wp.tile([128, 72, 128], mm_dtype, tag="wslab")
                    wsrc = wq[a, ct].rearrange("ci (j co) -> ci j co", co=128)
                    # per beta-half: first matmuls only wait for half
                    nc.sync.dma_start(wslab[:, 0:36, :], wsrc[:, 0:36, :])
                    nc.sync.dma_start(wslab[:, 36:72, :], wsrc[:, 36:72, :])
                for c in range(8):  # chunk of 8 output-phase rows
                    ot = op.tile([128, 8, 128], F32)
                    for b in range(2):   # output col parity (beta)
                        ps = pp.tile([128, 8, 64], F32)
                        for cg in range(4):
                            for tap in range(9):
                                ky, kx = tap // 3, tap % 3
                                j = b * 36 + cg * 9 + tap
                                nc.tensor.matmul(
                                    ps[:],
                                    wslab[:, j, :],
                                    xq[:, cg, 8 * c + ky:8 * c + ky + 8,
                                       kx:kx + 64],
                                    start=(cg == 0 and tap == 0),
                                    stop=(cg == 3 and tap == 8),
                                )
                        # psum += noise; z = psum + bias -> ot (strided by beta)
                        nc.vector.tensor_add(
                            ps[:], ps[:], noise_sb[:, b, 8 * c:8 * c + 8, :])
                        nc.scalar.activation(
                            ot[:, :, b::2], ps[:],
                            mybir.ActivationFunctionType.Identity,
                            bias=bias_sb[:, ct:ct + 1], scale=1.0)
                    # leaky relu: ot = max(0.2*ot, ot)
                    nc.vector.scalar_tensor_tensor(
                        ot[:], ot[:], 0.2, ot[:],
                        mybir.AluOpType.mult, mybir.AluOpType.max)
                    ydst = bass.AP(
                        tensor=y.tensor,
                        offset=(ct * 128) * 16384 + (16 * c + a) * 128,
                        ap=[[16384, 128], [256, 8], [1, 128]],
                    )
                    nc.sync.dma_start(ydst, ot[:])
    legalize_waits(nc)
    return nc


# ---------------------------------------------------------------------------
# Host-side preparation (weight renorm + FIR folding + phase decomposition)
# ---------------------------------------------------------------------------

def prep_inputs(x, weight, bias, noise_const, noise_strength):
    SQ2 = np.sqrt(2.0)
    w = np.asarray(weight).astype(np.float64)
    inv = 1.0 / np.sqrt((w ** 2).sum(axis=(1, 2, 3)) + 1e-8)
    w = w * inv[:, None, None, None]
    f = np.array([1., 3., 3., 1.])
    f = np.outer(f, f)
    f = f / f.sum() * 4.0                       # FIR * up^2 gain
    wf = w[:, :, ::-1, ::-1]                    # flipped (cross-corr of flip)
    g = np.zeros((w.shape[0], w.shape[1], 6, 6))
    for m in range(3):
        for n in range(3):
            g[:, :, m:m + 4, n:n + 4] += wf[:, :, m, n, None, None] * f
    # wq[a, ct, ci, b*36 + cg*9 + tap, co]
    wq = np.empty((2, 2, 128, 72, 128), dtype=np.float32)
    for a in range(2):
        for b in range(2):
            h = g[:, :, (1 - a)::2, (1 - b)::2] * SQ2    # [Cout, Cin, 3, 3]
            h5 = h.reshape(2, 128, 4, 128, 9)            # [ct, co, cg, ci, tap]
            h5 = h5.transpose(0, 3, 2, 4, 1)             # [ct, ci, cg, tap, co]
            wq[a, :, :, b * 36:(b + 1) * 36, :] = h5.reshape(2, 128, 36, 128)
    wq = np.ascontiguousarray(wq.reshape(2, 2, 128, 72 * 128), dtype=np.float32)

    noise2 = np.asarray(noise_const).astype(np.float64) * float(noise_strength) * SQ2
    noise4 = np.empty((2, 2, 64, 64), dtype=np.float32)
    for a in range(2):
        for b in range(2):
            noise4[a, b] = noise2[a::2, b::2]

    biasq = np.empty((128, 2), dtype=np.float32)
    bias2 = np.asarray(bias).astype(np.float64) * SQ2
    biasq[:, 0] = bias2[:128]
    biasq[:, 1] = bias2[128:]

    x = np.asarray(x)
    return [{
        "xin": np.ascontiguousarray(x[bi], dtype=np.float32),
        "wq": wq,
        "noise4": noise4,
        "biasq": biasq,
    } for bi in range(x.shape[0])]


_NC_CACHE = None


def kernel(x, weight, bias, noise_const, noise_strength):
    global _NC_CACHE
    if _NC_CACHE is None:
        _NC_CACHE = build_conv_nc()
    in_maps = prep_inputs(x, weight, bias, noise_const, noise_strength)
    res = run_bass_kernel_spmd(_NC_CACHE, in_maps, core_ids=list(range(8)))
    return np.ascontiguousarray(
        np.stack([r["y"] for r in res.results]), dtype=np.float32)



# revision 7
# speedup vs baseline: 1.6406x; 1.6406x over previous
"""Trainium2 Bass kernel for nn_Conv2d_61881888800824.

StyleGAN2-style synthesis layer:
    renorm(weight) -> upsample x2 (zero-insert) -> 4x4 FIR -> 3x3 conv
    -> + noise*strength -> + bias -> leaky_relu(0.2) * sqrt(2)

Math: the two convolutions commute on the upsampled grid, so the channel
mixing is done FIRST, at 64x64 resolution (9 taps total across 4 parity
phases of the upsampled grid = 4.83 GMAC/core, 4x less TensorE work than
folding the FIR into the conv), and the depthwise 4x4 FIR is applied
AFTERWARD as a separable 2-pass filter on the Vector engine in bf16.

    v[p',q'] = parity sub-kernels of wf (1/2/2/4 taps)  -- TensorE, bf16
    u[p',b]  = column FIR [1,3,3,1] of v                -- DVE (STT/TT)
    y[a,b]   = row FIR of u + noise (tap-1 fused)       -- DVE
    out      = Lrelu(y + bias) * sqrt2-folded           -- ScalarE, fused

Sharding: data-parallel over batch -- 1 image per NeuronCore, 8 cores.

Self-contained: inputs are the full tensors from setup_inputs(); output is
the full [8, 256, 128, 128] fp32 array.
"""
from contextlib import ExitStack

import numpy as np

import bass_rust
import concourse.bass as bass
import concourse.mybir as mybir
import concourse.tile as tile
from concourse.bass_utils import run_bass_kernel_spmd

F32 = mybir.dt.float32
BF16 = mybir.dt.bfloat16
MULT = mybir.AluOpType.mult
ADD = mybir.AluOpType.add

# (p', q') phases of the upsampled grid; per phase: list of
# (global_tap_idx, dy, dx) where v_ph[i,j] += W[tap] @ xpad[i+dy, j+dx]
PHASES = [
    ((0, 0), [(0, 0, 0)]),
    ((0, 1), [(1, 0, 0), (2, 0, 1)]),
    ((1, 0), [(3, 0, 0), (4, 1, 0)]),
    ((1, 1), [(5, 0, 0), (6, 0, 1), (7, 1, 0), (8, 1, 1)]),
]
# (u, v) index into wf for each global tap
TAPLIST = [(1, 1), (1, 0), (1, 2), (0, 1), (2, 1), (0, 0), (0, 2), (2, 0), (2, 2)]
ROWS = {0: 66, 1: 65}   # v rows per p'
COLS = {0: 66, 1: 65}   # v cols per q'
BND = 7                 # v-band rows (PSUM chunk = 7*66 fp32 <= 512)
NB = 10                 # ceil(66/7)
YBND = 16               # y-band rows
NYB = 4


# ---------------------------------------------------------------------------
# Wait legalization: this walrus build accepts at most ONE embedded sync wait
# per instruction. Tile can emit more (incl. same-engine self-waits that are
# provably satisfied by the engine's serial program order). Drop the provable
# ones; move the rest onto standalone EventSemaphore instructions inserted
# just before the over-limit instruction on the same engine.
# ---------------------------------------------------------------------------

def _is_async_update(inst) -> bool:
    n = type(inst).__name__
    return 'DMA' in n or 'Swdge' in n or 'Collective' in n or 'Dma' in n


def legalize_waits(nc, evsem_limit: int = 1) -> int:
    n_fixed = 0
    for fn in nc.m.functions:
        for bb in fn.blocks:
            insts = bb.instructions
            cum: dict[tuple, int] = {}
            out = []
            changed = False
            for inst in insts:
                si = inst.sync_info
                waits = list(si.on_wait) if si is not None and si.on_wait else []
                updates = list(si.on_update) if si is not None and si.on_update else []
                eng = inst.engine
                limit = 1
                if len(waits) > limit:
                    kept = []
                    for w in waits:
                        if (w.sync_type == 'semaphore'
                                and w.wait_mode == 'sem-ge-imm'
                                and w.wait_reg is None
                                and cum.get((eng, w.id), 0) >= w.wait_value):
                            continue
                        kept.append(w)
                    waits = kept
                if len(waits) > limit:
                    excess = waits[:-limit]
                    waits = waits[-limit:]
                    while excess:
                        take, excess = excess[:evsem_limit], excess[evsem_limit:]
                        ev = mybir.InstEventSemaphore(
                            name=nc.get_next_instruction_name(), ins=[], outs=[])
                        ev.engine = eng
                        ev.sync_info = bass_rust.SyncInfo(on_wait=take, on_update=[])
                        out.append(ev)
                    inst.sync_info = bass_rust.SyncInfo(on_wait=waits,
                                                        on_update=updates)
                    changed = True
                    n_fixed += 1
                elif si is not None and len(list(si.on_wait or [])) != len(waits):
                    inst.sync_info = bass_rust.SyncInfo(on_wait=waits,
                                                        on_update=updates)
                    changed = True
                    n_fixed += 1
                out.append(inst)
                if not _is_async_update(inst):
                    for u in updates:
                        if (u.sync_type == 'semaphore'
                                and u.update_mode == 'sem-inc'
                                and u.update_reg is None):
                            k = (eng, u.id)
                            cum[k] = cum.get(k, 0) + u.update_value
            if changed:
                bb.instructions = out
    return n_fixed


# ---------------------------------------------------------------------------
# Device kernel (per core: one batch image)
# ---------------------------------------------------------------------------

def build_conv_nc():
    nc = bass.Bass("TRN2", target_bir_lowering=False, debug=False)
    xin = nc.dram_tensor("xin", [4, 128, 64, 64], BF16, kind="ExternalInput").ap()
    wq = nc.dram_tensor("wq", [128, 4, 9, 2, 128], BF16, kind="ExternalInput").ap()
    noise4 = nc.dram_tensor("noise4", [4, 64, 64], BF16, kind="ExternalInput").ap()
    biasq = nc.dram_tensor("biasq", [128, 2], F32, kind="ExternalInput").ap()
    y4 = nc.dram_tensor("y4", [4, 2, 128, 64, 64], BF16, kind="ExternalOutput").ap()

    with ExitStack() as ctx:
        tc = ctx.enter_context(tile.TileContext(nc))
        xp = ctx.enter_context(tc.tile_pool(name="x", bufs=1))
        wp = ctx.enter_context(tc.tile_pool(name="w", bufs=1))
        np_ = ctx.enter_context(tc.tile_pool(name="noise", bufs=1))
        bp = ctx.enter_context(tc.tile_pool(name="bias", bufs=1))
        vp = ctx.enter_context(tc.tile_pool(name="v", bufs=3))
        up = ctx.enter_context(tc.tile_pool(name="u", bufs=1))
        tp = ctx.enter_context(tc.tile_pool(name="t", bufs=4))
        yp = ctx.enter_context(tc.tile_pool(name="y", bufs=2))
        op = ctx.enter_context(tc.tile_pool(name="o", bufs=2))
        pp = ctx.enter_context(tc.tile_pool(name="psum", bufs=8, space="PSUM"))

        bias_sb = bp.tile([128, 2], F32)
        nc.sync.dma_start(bias_sb[:], biasq)
        alpha_sb = bp.tile([128, 1], F32)
        nc.gpsimd.memset(alpha_sb[:], 0.2)

        # all 9-tap weights resident: [ci, cg, tap, ct, co]
        wslab = wp.tile([128, 4, 9, 2, 128], BF16)
        for cg in range(4):
            nc.sync.dma_start(wslab[:, cg], wq[:, cg])

        noise_sb = np_.tile([128, 4, 64, 64], BF16)
        nsrc = bass.AP(tensor=noise4.tensor, offset=0,
                       ap=[[0, 128], [4096, 4], [64, 64], [1, 64]])
        nc.sync.dma_start(noise_sb[:], nsrc)

        # x with 1-pixel zero border: [128, cg, 66, 66]
        xq = xp.tile([128, 4, 66, 66], BF16)
        for cg in range(4):
            nc.gpsimd.memset(xq[:, cg, 0, :], 0.0)
            nc.gpsimd.memset(xq[:, cg, 65, :], 0.0)
            nc.gpsimd.memset(xq[:, cg, 1:65, 0], 0.0)
            nc.gpsimd.memset(xq[:, cg, 1:65, 65], 0.0)
        for cg in range(4):
            for qtr in range(4):
                r = 16 * qtr
                nc.sync.dma_start(xq[:, cg, 1 + r:1 + r + 16, 1:65],
                                  xin[cg, :, r:r + 16, :])

        # full-resident u: [ci, p', b, ct, row, col]
        ut = up.tile([128, 2, 2, 2, 66, 64], BF16)

        yb_emitted = 0

        def emit_yband(m):
            i0 = YBND * m
            for a in range(2):
                for b in range(2):
                    phi = 2 * a + b
                    yt = yp.tile([128, 2, YBND, 64], BF16, tag="yt")
                    if a == 0:
                        taps = [(0, 0, 1.0), (1, 0, 3.0), (0, 1, 3.0), (1, 1, 1.0)]
                    else:
                        taps = [(1, 0, 1.0), (0, 1, 3.0), (1, 1, 3.0), (0, 2, 1.0)]
                    # tap 1 fuses the noise add (per-ct: noise has no ct dim)
                    p0, d0, c0 = taps[0]
                    for ct in range(2):
                        nc.vector.scalar_tensor_tensor(
                            yt[:, ct], ut[:, p0, b, ct, i0 + d0:i0 + d0 + YBND, :],
                            c0, noise_sb[:, phi, i0:i0 + YBND, :], MULT, ADD)
                    for p, d, c in taps[1:]:
                        nc.vector.scalar_tensor_tensor(
                            yt[:], ut[:, p, b, :, i0 + d:i0 + d + YBND, :],
                            c, yt[:], MULT, ADD)
                    ot = op.tile([128, 2, YBND, 64], BF16, tag="ot")
                    for ct in range(2):
                        nc.scalar.activation(
                            ot[:, ct], yt[:, ct],
                            mybir.ActivationFunctionType.Prelu,
                            bias=bias_sb[:, ct:ct + 1], scale=1.0,
                            alpha=alpha_sb[:, 0:1])
                        ydst = bass.AP(
                            tensor=y4.tensor,
                            offset=phi * 1048576 + ct * 524288 + i0 * 64,
                            ap=[[4096, 128], [64, YBND], [1, 64]])
                        nc.sync.dma_start(ydst, ot[:, ct])

        for k in range(NB):
            r0 = BND * k
            vt = vp.tile([128, 2, 4, BND, 66], BF16, tag="vt")
            for phi, ((pp_, qq), taps) in enumerate(PHASES):
                nr = min(BND, ROWS[pp_] - r0)
                cols = COLS[qq]
                if nr <= 0:
                    continue
                for ct in range(2):
                    ps = pp.tile([128, BND, 66], F32, tag="ps")
                    nt = len(taps)
                    for cg in range(4):
                        for it, (t, dy, dx) in enumerate(taps):
                            nc.tensor.matmul(
                                ps[:, 0:nr, 0:cols],
                                wslab[:, cg, t, ct, :],
                                xq[:, cg, r0 + dy:r0 + dy + nr, dx:dx + cols],
                                start=(cg == 0 and it == 0),
                                stop=(cg == 3 and it == nt - 1))
                    nc.scalar.copy(vt[:, ct, phi, 0:nr, 0:cols],
                                   ps[:, 0:nr, 0:cols])
            # column FIR: u[p', b] rows of this band
            for p in range(2):
                nr = min(BND, ROWS[p] - r0)
                if nr <= 0:
                    continue
                v0 = vt[:, :, 2 * p + 0]
                v1 = vt[:, :, 2 * p + 1]
                for b in range(2):
                    t1 = tp.tile([128, 2, BND, 64], BF16, tag="t1")
                    t2 = tp.tile([128, 2, BND, 64], BF16, tag="t2")
                    if b == 0:
                        nc.vector.tensor_add(t1[:, :, 0:nr, :],
                                             v0[:, :, 0:nr, 0:64],
                                             v1[:, :, 0:nr, 1:65])
                        nc.vector.tensor_add(t2[:, :, 0:nr, :],
                                             v1[:, :, 0:nr, 0:64],
                                             v0[:, :, 0:nr, 1:65])
                    else:
                        nc.vector.tensor_add(t1[:, :, 0:nr, :],
                                             v1[:, :, 0:nr, 0:64],
                                             v0[:, :, 0:nr, 2:66])
                        nc.vector.tensor_add(t2[:, :, 0:nr, :],
                                             v0[:, :, 0:nr, 1:65],
                                             v1[:, :, 0:nr, 1:65])
                    nc.vector.scalar_tensor_tensor(
                        ut[:, p, b, :, r0:r0 + nr, :],
                        t2[:, :, 0:nr, :], 3.0, t1[:, :, 0:nr, :], MULT, ADD)
            # row FIR + epilogue for y bands whose u rows are now complete.
            # y band m (a=1) reads u0 rows through 16m+17, u1 through 16m+16.
            done0 = min(r0 + BND, 66)
            done1 = min(r0 + BND, 65)
            while (yb_emitted < NYB
                   and done0 >= YBND * yb_emitted + YBND + 2
                   and done1 >= YBND * yb_emitted + YBND + 1):
                emit_yband(yb_emitted)
                yb_emitted += 1
        while yb_emitted < NYB:
            emit_yband(yb_emitted)
            yb_emitted += 1

    legalize_waits(nc)
    return nc


# ---------------------------------------------------------------------------
# Host-side preparation (renorm + flip + phase decomposition, all in fp64)
# ---------------------------------------------------------------------------

def prep_inputs(x, weight, bias, noise_const, noise_strength):
    from ml_dtypes import bfloat16
    SQ2 = np.sqrt(2.0)
    w = np.asarray(weight, np.float64)
    inv = 1.0 / np.sqrt((w ** 2).sum(axis=(1, 2, 3)) + 1e-8)
    wf = (w * inv[:, None, None, None])[:, :, ::-1, ::-1] * (SQ2 / 16.0)
    W = wf.transpose(1, 0, 2, 3)                 # [ci, co, u, v]

    wq = np.empty((128, 4, 9, 2, 128), np.float32)
    for t, (u, v) in enumerate(TAPLIST):
        for cg in range(4):
            for ct in range(2):
                wq[:, cg, t, ct, :] = W[cg * 128:(cg + 1) * 128,
                                        ct * 128:(ct + 1) * 128, u, v]
    wq = np.ascontiguousarray(wq.astype(bfloat16))

    s = float(np.asarray(noise_strength)) * SQ2
    nco = np.asarray(noise_const, np.float64)
    noise4 = np.empty((4, 64, 64), np.float32)
    for a in range(2):
        for b in range(2):
            noise4[2 * a + b] = nco[a::2, b::2] * s
    noise4 = np.ascontiguousarray(noise4.astype(bfloat16))

    biasq = np.empty((128, 2), np.float32)
    b64 = np.asarray(bias, np.float64) * SQ2
    biasq[:, 0] = b64[:128]
    biasq[:, 1] = b64[128:]

    xb = np.asarray(x, np.float32).astype(bfloat16)   # [8, 512, 64, 64]
    return [{
        "xin": np.ascontiguousarray(xb[i].reshape(4, 128, 64, 64)),
        "wq": wq,
        "noise4": noise4,
        "biasq": biasq,
    } for i in range(x.shape[0])]


_NC_CACHE = None


def kernel(x, weight, bias, noise_const, noise_strength):
    global _NC_CACHE
    if _NC_CACHE is None:
        _NC_CACHE = build_conv_nc()
    in_maps = prep_inputs(x, weight, bias, noise_const, noise_strength)
    res = run_bass_kernel_spmd(_NC_CACHE, in_maps, core_ids=list(range(8)))
    out = np.empty((x.shape[0], 256, 128, 128), np.float32)
    for i, r in enumerate(res.results):
        yq = np.asarray(r["y4"]).astype(np.float32)   # [4, 2, 128, 64, 64]
        for a in range(2):
            for b in range(2):
                out[i, :, a::2, b::2] = yq[2 * a + b].reshape(256, 64, 64)
    return out


# revision 11
# speedup vs baseline: 2.0020x; 1.2203x over previous
"""Trainium2 Bass kernel for nn_Conv2d_61881888800824.

StyleGAN2-style synthesis layer:
    renorm(weight) -> upsample x2 (zero-insert) -> 4x4 FIR -> 3x3 conv
    -> + noise*strength -> + bias -> leaky_relu(0.2) * sqrt(2)

Math: the two convolutions commute on the upsampled grid, so the channel
mixing is done FIRST, at 64x64 resolution (9 taps total across 4 parity
phases of the upsampled grid = 4.83 GMAC/core, 4x less TensorE work than
folding the FIR into the conv), and the depthwise 4x4 FIR is applied
AFTERWARD as a separable 2-pass filter in bf16.

    v[p',q'] = parity sub-kernels of wf (1/2/2/4 taps)  -- TensorE, bf16
    u[p',b]  = column FIR [1,3,3,1] of v                -- DVE TT/TS only
    y[a,b]   = row FIR of u                             -- DVE TT/TS only
    yt       = y + noise                                -- GpSimd TT
    out      = Prelu(yt + bias) (sqrt2 folded in w)     -- ScalarE, fused

All DVE work uses tensor_tensor (2x perf mode) and tensor_scalar (4x);
scalar_tensor_tensor has no DVE perf modes and is avoided.

Sharding: data-parallel over batch -- 1 image per NeuronCore, 8 cores.

Self-contained: inputs are the full tensors from setup_inputs(); output is
the full [8, 256, 128, 128] fp32 array.
"""
from contextlib import ExitStack

import numpy as np

import bass_rust
import concourse.bass as bass
import concourse.mybir as mybir
import concourse.tile as tile
from concourse.bass_utils import run_bass_kernel_spmd

F32 = mybir.dt.float32
BF16 = mybir.dt.bfloat16
MULT = mybir.AluOpType.mult
ADD = mybir.AluOpType.add

# (p', q') phases of the upsampled grid; per phase: list of
# (global_tap_idx, dy, dx) where v_ph[i,j] += W[tap] @ xpad[i+dy, j+dx]
PHASES = [
    ((0, 0), [(0, 0, 0)]),
    ((0, 1), [(1, 0, 0), (2, 0, 1)]),
    ((1, 0), [(3, 0, 0), (4, 1, 0)]),
    ((1, 1), [(5, 0, 0), (6, 0, 1), (7, 1, 0), (8, 1, 1)]),
]
# (u, v) index into wf for each global tap
TAPLIST = [(1, 1), (1, 0), (1, 2), (0, 1), (2, 1), (0, 0), (0, 2), (2, 0), (2, 2)]
ROWS = {0: 66, 1: 65}   # v rows per p'
COLS = {0: 66, 1: 65}   # v cols per q'
BND = 14                # v-band rows (PSUM tile = 14*66 fp32 = 2 banks)
NB = 5                  # ceil(66/14)
YBND = 16               # y-band rows
NYB = 4
# v-band after which y-band m's u rows are complete (see gating derivation)
YGATE = {0: 1, 1: 2, 2: 3, 3: 4}


def _is_async_update(inst) -> bool:
    n = type(inst).__name__
    return 'DMA' in n or 'Swdge' in n or 'Collective' in n or 'Dma' in n


def legalize_waits(nc, evsem_limit: int = 1) -> int:
    """This walrus build accepts at most ONE embedded sync wait per
    instruction. Drop provably-satisfied same-engine waits; move the rest
    onto standalone EventSemaphore instructions."""
    n_fixed = 0
    for fn in nc.m.functions:
        for bb in fn.blocks:
            insts = bb.instructions
            cum: dict[tuple, int] = {}
            out = []
            changed = False
            for inst in insts:
                si = inst.sync_info
                waits = list(si.on_wait) if si is not None and si.on_wait else []
                updates = list(si.on_update) if si is not None and si.on_update else []
                eng = inst.engine
                limit = 1
                if len(waits) > limit:
                    kept = []
                    for w in waits:
                        if (w.sync_type == 'semaphore'
                                and w.wait_mode == 'sem-ge-imm'
                                and w.wait_reg is None
                                and cum.get((eng, w.id), 0) >= w.wait_value):
                            continue
                        kept.append(w)
                    waits = kept
                if len(waits) > limit:
                    excess = waits[:-limit]
                    waits = waits[-limit:]
                    while excess:
                        take, excess = excess[:evsem_limit], excess[evsem_limit:]
                        ev = mybir.InstEventSemaphore(
                            name=nc.get_next_instruction_name(), ins=[], outs=[])
                        ev.engine = eng
                        ev.sync_info = bass_rust.SyncInfo(on_wait=take, on_update=[])
                        out.append(ev)
                    inst.sync_info = bass_rust.SyncInfo(on_wait=waits,
                                                        on_update=updates)
                    changed = True
                    n_fixed += 1
                elif si is not None and len(list(si.on_wait or [])) != len(waits):
                    inst.sync_info = bass_rust.SyncInfo(on_wait=waits,
                                                        on_update=updates)
                    changed = True
                    n_fixed += 1
                out.append(inst)
                if not _is_async_update(inst):
                    for u in updates:
                        if (u.sync_type == 'semaphore'
                                and u.update_mode == 'sem-inc'
                                and u.update_reg is None):
                            k = (eng, u.id)
                            cum[k] = cum.get(k, 0) + u.update_value
            if changed:
                bb.instructions = out
    return n_fixed


# ---------------------------------------------------------------------------
# Device kernel (per core: one batch image)
# ---------------------------------------------------------------------------

def build_conv_nc():
    nc = bass.Bass("TRN2", target_bir_lowering=False, debug=False)
    xin = nc.dram_tensor("xin", [4, 128, 64, 64], BF16, kind="ExternalInput").ap()
    wq = nc.dram_tensor("wq", [128, 4, 9, 2, 128], BF16, kind="ExternalInput").ap()
    noise4 = nc.dram_tensor("noise4", [4, 64, 64], BF16, kind="ExternalInput").ap()
    biasq = nc.dram_tensor("biasq", [128, 2], F32, kind="ExternalInput").ap()
    y4 = nc.dram_tensor("y4", [4, 2, 128, 64, 64], BF16, kind="ExternalOutput").ap()

    with ExitStack() as ctx:
        tc = ctx.enter_context(tile.TileContext(nc))
        xp = ctx.enter_context(tc.tile_pool(name="x", bufs=1))
        wp = ctx.enter_context(tc.tile_pool(name="w", bufs=1))
        np_ = ctx.enter_context(tc.tile_pool(name="noise", bufs=2))
        bp = ctx.enter_context(tc.tile_pool(name="bias", bufs=1))
        vp = ctx.enter_context(tc.tile_pool(name="v", bufs=2))
        up = ctx.enter_context(tc.tile_pool(name="u", bufs=1))
        tp = ctx.enter_context(tc.tile_pool(name="t", bufs=2))
        yp = ctx.enter_context(tc.tile_pool(name="y", bufs=1))
        op = ctx.enter_context(tc.tile_pool(name="o", bufs=2))
        pp = ctx.enter_context(tc.tile_pool(name="psum", bufs=8, space="PSUM"))

        bias_sb = bp.tile([128, 2], F32)
        nc.sync.dma_start(bias_sb[:], biasq)
        alpha_sb = bp.tile([128, 1], F32)
        nc.gpsimd.memset(alpha_sb[:], 0.2)

        # all 9-tap weights resident: [ci, cg, tap, ct, co]
        wslab = wp.tile([128, 4, 9, 2, 128], BF16)
        for cg in range(4):
            nc.sync.dma_start(wslab[:, cg], wq[:, cg])

        # x with 1-pixel zero border: [128, cg, 66, 66]
        xq = xp.tile([128, 4, 66, 66], BF16)
        for cg in range(4):
            nc.gpsimd.memset(xq[:, cg, 0, :], 0.0)
            nc.gpsimd.memset(xq[:, cg, 65, :], 0.0)
            nc.gpsimd.memset(xq[:, cg, 1:65, 0], 0.0)
            nc.gpsimd.memset(xq[:, cg, 1:65, 65], 0.0)
        for cg in range(4):
            for hf in range(2):
                r = 32 * hf
                nc.sync.dma_start(xq[:, cg, 1 + r:1 + r + 32, 1:65],
                                  xin[cg, :, r:r + 32, :])

        # full-resident u: [ci, p', b, ct, row, col]
        ut = up.tile([128, 2, 2, 2, 66, 64], BF16)

        # per-yband noise slabs, broadcast to all partitions: [128, ph, 16, 64]
        noise_t = {}

        def emit_noise(m):
            nt_ = np_.tile([128, 4, YBND, 64], BF16, tag="nt")
            nsrc = bass.AP(tensor=noise4.tensor, offset=YBND * m * 64,
                           ap=[[0, 128], [4096, 4], [64, YBND], [1, 64]])
            nc.sync.dma_start(nt_[:], nsrc)
            noise_t[m] = nt_

        emit_noise(0)
        emit_noise(1)

        # deferred per-yband work
        yt_tiles = {}

        def emit_ytaps(m):
            i0 = YBND * m
            yt = yp.tile([128, 4, 2, YBND, 64], BF16, tag="yt")
            for a in range(2):
                for b in range(2):
                    phi = 2 * a + b
                    u0 = ut[:, 0, b]
                    u1 = ut[:, 1, b]
                    s1 = tp.tile([128, 2, YBND, 64], BF16, tag="s1", bufs=2)
                    s2 = tp.tile([128, 2, YBND, 64], BF16, tag="s2", bufs=1)
                    if a == 0:
                        # y = (u0[i] + u1[i+1]) + 3*(u1[i] + u0[i+1])
                        nc.vector.tensor_tensor(
                            s1[:], u0[:, :, i0:i0 + YBND, :],
                            u1[:, :, i0 + 1:i0 + 1 + YBND, :], ADD)
                        nc.vector.tensor_tensor(
                            s2[:], u1[:, :, i0:i0 + YBND, :],
                            u0[:, :, i0 + 1:i0 + 1 + YBND, :], ADD)
                    else:
                        # y = (u1[i] + u0[i+2]) + 3*(u0[i+1] + u1[i+1])
                        nc.vector.tensor_tensor(
                            s1[:], u1[:, :, i0:i0 + YBND, :],
                            u0[:, :, i0 + 2:i0 + 2 + YBND, :], ADD)
                        nc.vector.tensor_tensor(
                            s2[:], u0[:, :, i0 + 1:i0 + 1 + YBND, :],
                            u1[:, :, i0 + 1:i0 + 1 + YBND, :], ADD)
                    nc.vector.tensor_scalar(s2[:], s2[:], 3.0, None, MULT)
                    nc.vector.tensor_tensor(s1[:], s1[:], s2[:], ADD)
                    # + noise on GpSimd (per ct: noise has no ct dim)
                    for ct in range(2):
                        nc.gpsimd.tensor_tensor(
                            yt[:, phi, ct], s1[:, ct],
                            noise_t[m][:, phi], ADD)
            yt_tiles[m] = yt

        def emit_yacts(m):
            i0 = YBND * m
            yt = yt_tiles.pop(m)
            for phi in range(4):
                ot = op.tile([128, 2, YBND, 64], BF16, tag="ot")
                for ct in range(2):
                    nc.scalar.activation(
                        ot[:, ct], yt[:, phi, ct],
                        mybir.ActivationFunctionType.Prelu,
                        bias=bias_sb[:, ct:ct + 1], scale=1.0,
                        alpha=alpha_sb[:, 0:1])
                ydst = bass.AP(
                    tensor=y4.tensor, offset=phi * 1048576 + i0 * 64,
                    ap=[[4096, 128], [524288, 2], [64, YBND], [1, 64]])
                nc.sync.dma_start(ydst, ot[:])

        taps_done = 0
        acts_done = 0
        for k in range(NB):
            r0 = BND * k
            vt = vp.tile([128, 2, 4, BND, 66], BF16, tag="vt")
            for phi, ((pp_, qq), taps) in enumerate(PHASES):
                nr = min(BND, ROWS[pp_] - r0)
                cols = COLS[qq]
                halves = [(0, min(7, nr))]
                if nr > 7:
                    halves.append((7, nr - 7))
                for ct in range(2):
                    pst = [pp.tile([128, 7, 66], F32, tag="ps", name=f"ps{hh}")
                           for hh, _ in enumerate(halves)]
                    nt = len(taps)
                    for cg in range(4):
                        for it, (t, dy, dx) in enumerate(taps):
                            first = (cg == 0 and it == 0)
                            last = (cg == 3 and it == nt - 1)
                            for (h, hn), ps in zip(halves, pst):
                                nc.tensor.matmul(
                                    ps[:, 0:hn, 0:cols],
                                    wslab[:, cg, t, ct, :],
                                    xq[:, cg,
                                       r0 + h + dy:r0 + h + dy + hn,
                                       dx:dx + cols],
                                    start=first, stop=last,
                                    skip_group_check=True)
                    for (h, hn), ps in zip(halves, pst):
                        nc.scalar.copy(vt[:, ct, phi, h:h + hn, 0:cols],
                                       ps[:, 0:hn, 0:cols])
            # column FIR: u[p', b] rows of this band (no row halo needed)
            for p in range(2):
                nr = min(BND, ROWS[p] - r0)
                if nr <= 0:
                    continue
                v0 = vt[:, :, 2 * p + 0]
                v1 = vt[:, :, 2 * p + 1]
                for b in range(2):
                    usl = ut[:, p, b, :, r0:r0 + nr, :]
                    s2 = tp.tile([128, 2, BND, 64], BF16, tag="us2", bufs=1)
                    if b == 0:
                        # u = (v0[j] + v1[j+1]) + 3*(v1[j] + v0[j+1])
                        nc.vector.tensor_tensor(usl,
                                                v0[:, :, 0:nr, 0:64],
                                                v1[:, :, 0:nr, 1:65], ADD)
                        nc.vector.tensor_tensor(s2[:, :, 0:nr, :],
                                                v1[:, :, 0:nr, 0:64],
                                                v0[:, :, 0:nr, 1:65], ADD)
                    else:
                        # u = (v1[j] + v0[j+2]) + 3*(v0[j+1] + v1[j+1])
                        nc.vector.tensor_tensor(usl,
                                                v1[:, :, 0:nr, 0:64],
                                                v0[:, :, 0:nr, 2:66], ADD)
                        nc.vector.tensor_tensor(s2[:, :, 0:nr, :],
                                                v0[:, :, 0:nr, 1:65],
                                                v1[:, :, 0:nr, 1:65], ADD)
                    nc.vector.tensor_scalar(s2[:, :, 0:nr, :],
                                            s2[:, :, 0:nr, :], 3.0, None, MULT)
                    nc.vector.tensor_tensor(usl, usl,
                                            s2[:, :, 0:nr, :], ADD)
            # deferred epilogue for the previous yband, then new ytaps
            while acts_done < taps_done:
                emit_yacts(acts_done)
                acts_done += 1
            while taps_done < NYB and YGATE[taps_done] <= k:
                emit_ytaps(taps_done)
                taps_done += 1
                if taps_done + 1 < NYB and (taps_done + 1) not in noise_t:
                    emit_noise(taps_done + 1)
        while acts_done < NYB:
            if taps_done < NYB:
                emit_ytaps(taps_done)
                taps_done += 1
            emit_yacts(acts_done)
            acts_done += 1

    legalize_waits(nc)
    return nc


# ---------------------------------------------------------------------------
# Host-side preparation (renorm + flip + phase decomposition, all in fp64)
# ---------------------------------------------------------------------------

def prep_inputs(x, weight, bias, noise_const, noise_strength):
    from ml_dtypes import bfloat16
    SQ2 = np.sqrt(2.0)
    w = np.asarray(weight, np.float64)
    inv = 1.0 / np.sqrt((w ** 2).sum(axis=(1, 2, 3)) + 1e-8)
    wf = (w * inv[:, None, None, None])[:, :, ::-1, ::-1] * (SQ2 / 16.0)
    W = wf.transpose(1, 0, 2, 3)                 # [ci, co, u, v]

    wq = np.empty((128, 4, 9, 2, 128), np.float32)
    for t, (u, v) in enumerate(TAPLIST):
        for cg in range(4):
            for ct in range(2):
                wq[:, cg, t, ct, :] = W[cg * 128:(cg + 1) * 128,
                                        ct * 128:(ct + 1) * 128, u, v]
    wq = np.ascontiguousarray(wq.astype(bfloat16))

    s = float(np.asarray(noise_strength)) * SQ2
    nco = np.asarray(noise_const, np.float64)
    noise4 = np.empty((4, 64, 64), np.float32)
    for a in range(2):
        for b in range(2):
            noise4[2 * a + b] = nco[a::2, b::2] * s
    noise4 = np.ascontiguousarray(noise4.astype(bfloat16))

    biasq = np.empty((128, 2), np.float32)
    b64 = np.asarray(bias, np.float64) * SQ2
    biasq[:, 0] = b64[:128]
    biasq[:, 1] = b64[128:]

    xb = np.asarray(x, np.float32).astype(bfloat16)   # [8, 512, 64, 64]
    return [{
        "xin": np.ascontiguousarray(xb[i].reshape(4, 128, 64, 64)),
        "wq": wq,
        "noise4": noise4,
        "biasq": biasq,
    } for i in range(x.shape[0])]


_NC_CACHE = None


def kernel(x, weight, bias, noise_const, noise_strength):
    global _NC_CACHE
    if _NC_CACHE is None:
        _NC_CACHE = build_conv_nc()
    in_maps = prep_inputs(x, weight, bias, noise_const, noise_strength)
    res = run_bass_kernel_spmd(_NC_CACHE, in_maps, core_ids=list(range(8)))
    out = np.empty((x.shape[0], 256, 128, 128), np.float32)
    for i, r in enumerate(res.results):
        yq = np.asarray(r["y4"]).astype(np.float32)   # [4, 2, 128, 64, 64]
        for a in range(2):
            for b in range(2):
                out[i, :, a::2, b::2] = yq[2 * a + b].reshape(256, 64, 64)
    return out


# revision 12
# speedup vs baseline: 2.3636x; 1.1806x over previous
"""Trainium2 Bass kernel for nn_Conv2d_61881888800824.

StyleGAN2-style synthesis layer:
    renorm(weight) -> upsample x2 (zero-insert) -> 4x4 FIR -> 3x3 conv
    -> + noise*strength -> + bias -> leaky_relu(0.2) * sqrt(2)

Math: the two convolutions commute on the upsampled grid, so the channel
mixing is done FIRST, at 64x64 resolution (9 taps total across 4 parity
phases of the upsampled grid = 4.83 GMAC/core, 4x less TensorE work than
folding the FIR into the conv), and the depthwise 4x4 FIR is applied
AFTERWARD as a separable 2-pass filter in bf16.

    v[p',q'] = parity sub-kernels of wf (1/2/2/4 taps)  -- TensorE, bf16
    u[p',b]  = column FIR [1,3,3,1] of v                -- DVE TT/TS only
    y[a,b]   = row FIR of u                             -- DVE TT/TS only
    yt       = y + noise                                -- GpSimd TT
    out      = Prelu(yt + bias) (sqrt2 folded in w)     -- ScalarE, fused

All DVE work uses tensor_tensor (2x perf mode) and tensor_scalar (4x);
scalar_tensor_tensor has no DVE perf modes and is avoided.

Sharding: data-parallel over batch -- 1 image per NeuronCore, 8 cores.

Self-contained: inputs are the full tensors from setup_inputs(); output is
the full [8, 256, 128, 128] fp32 array.
"""
from contextlib import ExitStack

import numpy as np

import bass_rust
import concourse.bass as bass
import concourse.mybir as mybir
import concourse.tile as tile
from concourse.bass_utils import run_bass_kernel_spmd

F32 = mybir.dt.float32
BF16 = mybir.dt.bfloat16
MULT = mybir.AluOpType.mult
ADD = mybir.AluOpType.add

# (p', q') phases of the upsampled grid; per phase: list of
# (global_tap_idx, dy, dx) where v_ph[i,j] += W[tap] @ xpad[i+dy, j+dx]
PHASES = [
    ((0, 0), [(0, 0, 0)]),
    ((0, 1), [(1, 0, 0), (2, 0, 1)]),
    ((1, 0), [(3, 0, 0), (4, 1, 0)]),
    ((1, 1), [(5, 0, 0), (6, 0, 1), (7, 1, 0), (8, 1, 1)]),
]
# (u, v) index into wf for each global tap
TAPLIST = [(1, 1), (1, 0), (1, 2), (0, 1), (2, 1), (0, 0), (0, 2), (2, 0), (2, 2)]
ROWS = {0: 66, 1: 65}   # v rows per p'
COLS = {0: 66, 1: 65}   # v cols per q'
BND = 14                # v-band rows (PSUM tile = 14*66 fp32 = 2 banks)
NB = 5                  # ceil(66/14)
YBND = 16               # y-band rows
NYB = 4
# v-band after which y-band m's u rows are complete (see gating derivation)
YGATE = {0: 1, 1: 2, 2: 3, 3: 4}


def _is_async_update(inst) -> bool:
    n = type(inst).__name__
    return 'DMA' in n or 'Swdge' in n or 'Collective' in n or 'Dma' in n


def legalize_waits(nc, evsem_limit: int = 1) -> int:
    """This walrus build accepts at most ONE embedded sync wait per
    instruction. Drop provably-satisfied same-engine waits; move the rest
    onto standalone EventSemaphore instructions."""
    n_fixed = 0
    for fn in nc.m.functions:
        for bb in fn.blocks:
            insts = bb.instructions
            cum: dict[tuple, int] = {}
            out = []
            changed = False
            for inst in insts:
                si = inst.sync_info
                waits = list(si.on_wait) if si is not None and si.on_wait else []
                updates = list(si.on_update) if si is not None and si.on_update else []
                eng = inst.engine
                limit = 1
                if len(waits) > limit:
                    kept = []
                    for w in waits:
                        if (w.sync_type == 'semaphore'
                                and w.wait_mode == 'sem-ge-imm'
                                and w.wait_reg is None
                                and cum.get((eng, w.id), 0) >= w.wait_value):
                            continue
                        kept.append(w)
                    waits = kept
                if len(waits) > limit:
                    excess = waits[:-limit]
                    waits = waits[-limit:]
                    while excess:
                        take, excess = excess[:evsem_limit], excess[evsem_limit:]
                        ev = mybir.InstEventSemaphore(
                            name=nc.get_next_instruction_name(), ins=[], outs=[])
                        ev.engine = eng
                        ev.sync_info = bass_rust.SyncInfo(on_wait=take, on_update=[])
                        out.append(ev)
                    inst.sync_info = bass_rust.SyncInfo(on_wait=waits,
                                                        on_update=updates)
                    changed = True
                    n_fixed += 1
                elif si is not None and len(list(si.on_wait or [])) != len(waits):
                    inst.sync_info = bass_rust.SyncInfo(on_wait=waits,
                                                        on_update=updates)
                    changed = True
                    n_fixed += 1
                out.append(inst)
                if not _is_async_update(inst):
                    for u in updates:
                        if (u.sync_type == 'semaphore'
                                and u.update_mode == 'sem-inc'
                                and u.update_reg is None):
                            k = (eng, u.id)
                            cum[k] = cum.get(k, 0) + u.update_value
            if changed:
                bb.instructions = out
    return n_fixed


# ---------------------------------------------------------------------------
# Device kernel (per core: one batch image)
# ---------------------------------------------------------------------------

def build_conv_nc():
    nc = bass.Bass("TRN2", target_bir_lowering=False, debug=False)
    xin = nc.dram_tensor("xin", [4, 128, 64, 64], BF16, kind="ExternalInput").ap()
    wq = nc.dram_tensor("wq", [128, 4, 9, 2, 128], BF16, kind="ExternalInput").ap()
    noise4 = nc.dram_tensor("noise4", [4, 64, 64], BF16, kind="ExternalInput").ap()
    biasq = nc.dram_tensor("biasq", [128, 2], F32, kind="ExternalInput").ap()
    y4 = nc.dram_tensor("y4", [4, 2, 128, 64, 64], BF16, kind="ExternalOutput").ap()

    with ExitStack() as ctx:
        tc = ctx.enter_context(tile.TileContext(nc))
        xp = ctx.enter_context(tc.tile_pool(name="x", bufs=1))
        wp = ctx.enter_context(tc.tile_pool(name="w", bufs=1))
        np_ = ctx.enter_context(tc.tile_pool(name="noise", bufs=2))
        bp = ctx.enter_context(tc.tile_pool(name="bias", bufs=1))
        vp = ctx.enter_context(tc.tile_pool(name="v", bufs=2))
        up = ctx.enter_context(tc.tile_pool(name="u", bufs=1))
        tp = ctx.enter_context(tc.tile_pool(name="t", bufs=2))
        yp = ctx.enter_context(tc.tile_pool(name="y", bufs=1))
        op = ctx.enter_context(tc.tile_pool(name="o", bufs=2))
        pp = ctx.enter_context(tc.tile_pool(name="psum", bufs=8, space="PSUM"))

        bias_sb = bp.tile([128, 2], F32)
        nc.sync.dma_start(bias_sb[:], biasq)
        alpha_sb = bp.tile([128, 1], F32)
        nc.gpsimd.memset(alpha_sb[:], 0.2)

        # all 9-tap weights resident: [ci, cg, tap, ct, co]
        wslab = wp.tile([128, 4, 9, 2, 128], BF16)
        for cg in range(4):
            nc.sync.dma_start(wslab[:, cg], wq[:, cg])

        # x with 1-pixel zero border: [128, cg, 66, 66]
        xq = xp.tile([128, 4, 66, 66], BF16)
        for cg in range(4):
            nc.gpsimd.memset(xq[:, cg, 0, :], 0.0)
            nc.gpsimd.memset(xq[:, cg, 65, :], 0.0)
            nc.gpsimd.memset(xq[:, cg, 1:65, 0], 0.0)
            nc.gpsimd.memset(xq[:, cg, 1:65, 65], 0.0)
        for cg in range(4):
            for hf in range(2):
                r = 32 * hf
                nc.sync.dma_start(xq[:, cg, 1 + r:1 + r + 32, 1:65],
                                  xin[cg, :, r:r + 32, :])

        # full-resident u: [ci, p', b, ct, row, col]
        ut = up.tile([128, 2, 2, 2, 66, 64], BF16)

        # per-yband noise slabs, broadcast to all partitions: [128, ph, 16, 64]
        noise_t = {}

        def emit_noise(m):
            nt_ = np_.tile([128, 4, YBND, 64], BF16, tag="nt")
            nsrc = bass.AP(tensor=noise4.tensor, offset=YBND * m * 64,
                           ap=[[0, 128], [4096, 4], [64, YBND], [1, 64]])
            nc.sync.dma_start(nt_[:], nsrc)
            noise_t[m] = nt_

        emit_noise(0)
        emit_noise(1)

        # deferred per-yband work
        yt_tiles = {}

        def emit_ytaps(m):
            i0 = YBND * m
            yt = yp.tile([128, 4, 2, YBND, 64], BF16, tag="yt")
            for a in range(2):
                for b in range(2):
                    phi = 2 * a + b
                    u0 = ut[:, 0, b]
                    u1 = ut[:, 1, b]
                    s1 = tp.tile([128, 2, YBND, 64], BF16, tag="s1", bufs=2)
                    s2 = tp.tile([128, 2, YBND, 64], BF16, tag="s2", bufs=1)
                    if a == 0:
                        # y = (u0[i] + u1[i+1]) + 3*(u1[i] + u0[i+1])
                        nc.vector.tensor_tensor(
                            s1[:], u0[:, :, i0:i0 + YBND, :],
                            u1[:, :, i0 + 1:i0 + 1 + YBND, :], ADD)
                        nc.vector.tensor_tensor(
                            s2[:], u1[:, :, i0:i0 + YBND, :],
                            u0[:, :, i0 + 1:i0 + 1 + YBND, :], ADD)
                    else:
                        # y = (u1[i] + u0[i+2]) + 3*(u0[i+1] + u1[i+1])
                        nc.vector.tensor_tensor(
                            s1[:], u1[:, :, i0:i0 + YBND, :],
                            u0[:, :, i0 + 2:i0 + 2 + YBND, :], ADD)
                        nc.vector.tensor_tensor(
                            s2[:], u0[:, :, i0 + 1:i0 + 1 + YBND, :],
                            u1[:, :, i0 + 1:i0 + 1 + YBND, :], ADD)
                    nc.vector.tensor_scalar(s2[:], s2[:], 3.0, None, MULT)
                    nc.vector.tensor_tensor(s1[:], s1[:], s2[:], ADD)
                    # + noise (per ct: noise has no ct dim)
                    for ct in range(2):
                        nc.vector.tensor_tensor(
                            yt[:, phi, ct], s1[:, ct],
                            noise_t[m][:, phi], ADD)
            yt_tiles[m] = yt

        def emit_yacts(m):
            i0 = YBND * m
            yt = yt_tiles.pop(m)
            for phi in range(4):
                ot = op.tile([128, 2, YBND, 64], BF16, tag="ot")
                for ct in range(2):
                    nc.scalar.activation(
                        ot[:, ct], yt[:, phi, ct],
                        mybir.ActivationFunctionType.Prelu,
                        bias=bias_sb[:, ct:ct + 1], scale=1.0,
                        alpha=alpha_sb[:, 0:1])
                ydst = bass.AP(
                    tensor=y4.tensor, offset=phi * 1048576 + i0 * 64,
                    ap=[[4096, 128], [524288, 2], [64, YBND], [1, 64]])
                nc.sync.dma_start(ydst, ot[:])

        taps_done = 0
        acts_done = 0
        for k in range(NB):
            r0 = BND * k
            vt = vp.tile([128, 2, 4, BND, 66], BF16, tag="vt")
            for phi, ((pp_, qq), taps) in enumerate(PHASES):
                nr = min(BND, ROWS[pp_] - r0)
                cols = COLS[qq]
                halves = [(0, min(7, nr))]
                if nr > 7:
                    halves.append((7, nr - 7))
                for ct in range(2):
                    pst = [pp.tile([128, 7, 66], F32, tag="ps", name=f"ps{hh}")
                           for hh, _ in enumerate(halves)]
                    nt = len(taps)
                    for cg in range(4):
                        for it, (t, dy, dx) in enumerate(taps):
                            first = (cg == 0 and it == 0)
                            last = (cg == 3 and it == nt - 1)
                            for (h, hn), ps in zip(halves, pst):
                                nc.tensor.matmul(
                                    ps[:, 0:hn, 0:cols],
                                    wslab[:, cg, t, ct, :],
                                    xq[:, cg,
                                       r0 + h + dy:r0 + h + dy + hn,
                                       dx:dx + cols],
                                    start=first, stop=last,
                                    skip_group_check=True)
                    for (h, hn), ps in zip(halves, pst):
                        nc.scalar.copy(vt[:, ct, phi, h:h + hn, 0:cols],
                                       ps[:, 0:hn, 0:cols])
            # column FIR: u[p', b] rows of this band (no row halo needed)
            for p in range(2):
                nr = min(BND, ROWS[p] - r0)
                if nr <= 0:
                    continue
                v0 = vt[:, :, 2 * p + 0]
                v1 = vt[:, :, 2 * p + 1]
                for b in range(2):
                    usl = ut[:, p, b, :, r0:r0 + nr, :]
                    s2 = tp.tile([128, 2, BND, 64], BF16, tag="us2", bufs=1)
                    if b == 0:
                        # u = (v0[j] + v1[j+1]) + 3*(v1[j] + v0[j+1])
                        nc.vector.tensor_tensor(usl,
                                                v0[:, :, 0:nr, 0:64],
                                                v1[:, :, 0:nr, 1:65], ADD)
                        nc.vector.tensor_tensor(s2[:, :, 0:nr, :],
                                                v1[:, :, 0:nr, 0:64],
                                                v0[:, :, 0:nr, 1:65], ADD)
                    else:
                        # u = (v1[j] + v0[j+2]) + 3*(v0[j+1] + v1[j+1])
                        nc.vector.tensor_tensor(usl,
                                                v1[:, :, 0:nr, 0:64],
                                                v0[:, :, 0:nr, 2:66], ADD)
                        nc.vector.tensor_tensor(s2[:, :, 0:nr, :],
                                                v0[:, :, 0:nr, 1:65],
                                                v1[:, :, 0:nr, 1:65], ADD)
                    nc.vector.tensor_scalar(s2[:, :, 0:nr, :],
                                            s2[:, :, 0:nr, :], 3.0, None, MULT)
                    nc.vector.tensor_tensor(usl, usl,
                                            s2[:, :, 0:nr, :], ADD)
            # deferred epilogue for the previous yband, then new ytaps
            while acts_done < taps_done:
                emit_yacts(acts_done)
                acts_done += 1
            while taps_done < NYB and YGATE[taps_done] <= k:
                emit_ytaps(taps_done)
                taps_done += 1
                if taps_done + 1 < NYB and (taps_done + 1) not in noise_t:
                    emit_noise(taps_done + 1)
        while acts_done < NYB:
            if taps_done < NYB:
                emit_ytaps(taps_done)
                taps_done += 1
            emit_yacts(acts_done)
            acts_done += 1

    legalize_waits(nc)
    return nc


# ---------------------------------------------------------------------------
# Host-side preparation (renorm + flip + phase decomposition, all in fp64)
# ---------------------------------------------------------------------------

def prep_inputs(x, weight, bias, noise_const, noise_strength):
    from ml_dtypes import bfloat16
    SQ2 = np.sqrt(2.0)
    w = np.asarray(weight, np.float64)
    inv = 1.0 / np.sqrt((w ** 2).sum(axis=(1, 2, 3)) + 1e-8)
    wf = (w * inv[:, None, None, None])[:, :, ::-1, ::-1] * (SQ2 / 16.0)
    W = wf.transpose(1, 0, 2, 3)                 # [ci, co, u, v]

    wq = np.empty((128, 4, 9, 2, 128), np.float32)
    for t, (u, v) in enumerate(TAPLIST):
        for cg in range(4):
            for ct in range(2):
                wq[:, cg, t, ct, :] = W[cg * 128:(cg + 1) * 128,
                                        ct * 128:(ct + 1) * 128, u, v]
    wq = np.ascontiguousarray(wq.astype(bfloat16))

    s = float(np.asarray(noise_strength)) * SQ2
    nco = np.asarray(noise_const, np.float64)
    noise4 = np.empty((4, 64, 64), np.float32)
    for a in range(2):
        for b in range(2):
            noise4[2 * a + b] = nco[a::2, b::2] * s
    noise4 = np.ascontiguousarray(noise4.astype(bfloat16))

    biasq = np.empty((128, 2), np.float32)
    b64 = np.asarray(bias, np.float64) * SQ2
    biasq[:, 0] = b64[:128]
    biasq[:, 1] = b64[128:]

    xb = np.asarray(x, np.float32).astype(bfloat16)   # [8, 512, 64, 64]
    return [{
        "xin": np.ascontiguousarray(xb[i].reshape(4, 128, 64, 64)),
        "wq": wq,
        "noise4": noise4,
        "biasq": biasq,
    } for i in range(x.shape[0])]


_NC_CACHE = None


def kernel(x, weight, bias, noise_const, noise_strength):
    global _NC_CACHE
    if _NC_CACHE is None:
        _NC_CACHE = build_conv_nc()
    in_maps = prep_inputs(x, weight, bias, noise_const, noise_strength)
    res = run_bass_kernel_spmd(_NC_CACHE, in_maps, core_ids=list(range(8)))
    out = np.empty((x.shape[0], 256, 128, 128), np.float32)
    for i, r in enumerate(res.results):
        yq = np.asarray(r["y4"]).astype(np.float32)   # [4, 2, 128, 64, 64]
        for a in range(2):
            for b in range(2):
                out[i, :, a::2, b::2] = yq[2 * a + b].reshape(256, 64, 64)
    return out


# revision 13
# speedup vs baseline: 2.6968x; 1.1410x over previous
"""Trainium2 Bass kernel for nn_Conv2d_61881888800824.

StyleGAN2-style synthesis layer:
    renorm(weight) -> upsample x2 (zero-insert) -> 4x4 FIR -> 3x3 conv
    -> + noise*strength -> + bias -> leaky_relu(0.2) * sqrt(2)

Math: the two convolutions commute on the upsampled grid, so the channel
mixing is done FIRST, at 64x64 resolution (9 taps total across 4 parity
phases of the upsampled grid = 4.83 GMAC/core, 4x less TensorE work than
folding the FIR into the conv), and the depthwise 4x4 FIR is applied
AFTERWARD as a separable 2-pass filter in bf16.

    v[p',q'] = parity sub-kernels of wf (1/2/2/4 taps)  -- TensorE, bf16
    u[p',b]  = column FIR [1,3,3,1] of v                -- DVE TT/TS only
    y[a,b]   = row FIR of u                             -- DVE TT/TS only
    yt       = y + noise                                -- GpSimd TT
    out      = Prelu(yt + bias) (sqrt2 folded in w)     -- ScalarE, fused

All DVE work uses tensor_tensor (2x perf mode) and tensor_scalar (4x);
scalar_tensor_tensor has no DVE perf modes and is avoided.

Sharding: data-parallel over batch -- 1 image per NeuronCore, 8 cores.

Self-contained: inputs are the full tensors from setup_inputs(); output is
the full [8, 256, 128, 128] fp32 array.
"""
from contextlib import ExitStack

import numpy as np

import bass_rust
import concourse.bass as bass
import concourse.mybir as mybir
import concourse.tile as tile
from concourse.bass_utils import run_bass_kernel_spmd

F32 = mybir.dt.float32
BF16 = mybir.dt.bfloat16
MULT = mybir.AluOpType.mult
ADD = mybir.AluOpType.add

# (p', q') phases of the upsampled grid; per phase: list of
# (global_tap_idx, dy, dx) where v_ph[i,j] += W[tap] @ xpad[i+dy, j+dx]
PHASES = [
    ((0, 0), [(0, 0, 0)]),
    ((0, 1), [(1, 0, 0), (2, 0, 1)]),
    ((1, 0), [(3, 0, 0), (4, 1, 0)]),
    ((1, 1), [(5, 0, 0), (6, 0, 1), (7, 1, 0), (8, 1, 1)]),
]
# (u, v) index into wf for each global tap
TAPLIST = [(1, 1), (1, 0), (1, 2), (0, 1), (2, 1), (0, 0), (0, 2), (2, 0), (2, 2)]
ROWS = {0: 66, 1: 65}   # v rows per p'
COLS = {0: 66, 1: 65}   # v cols per q'
BND = 14                # v-band rows (PSUM tile = 14*66 fp32 = 2 banks)
NB = 5                  # ceil(66/14)
YBND = 8                # y-band rows
NYB = 8
# v-band after which y-band m's u rows are complete:
# done0 >= 8m+10, done1 >= 8m+9 with done = (14,28,42,56,66/65)
YGATE = {0: 0, 1: 1, 2: 1, 3: 2, 4: 2, 5: 3, 6: 4, 7: 4}


def _is_async_update(inst) -> bool:
    n = type(inst).__name__
    return 'DMA' in n or 'Swdge' in n or 'Collective' in n or 'Dma' in n


def legalize_waits(nc, evsem_limit: int = 1) -> int:
    """This walrus build accepts at most ONE embedded sync wait per
    instruction. Drop provably-satisfied same-engine waits; move the rest
    onto standalone EventSemaphore instructions."""
    n_fixed = 0
    for fn in nc.m.functions:
        for bb in fn.blocks:
            insts = bb.instructions
            cum: dict[tuple, int] = {}
            out = []
            changed = False
            for inst in insts:
                si = inst.sync_info
                waits = list(si.on_wait) if si is not None and si.on_wait else []
                updates = list(si.on_update) if si is not None and si.on_update else []
                eng = inst.engine
                limit = 1
                if len(waits) > limit:
                    kept = []
                    for w in waits:
                        if (w.sync_type == 'semaphore'
                                and w.wait_mode == 'sem-ge-imm'
                                and w.wait_reg is None
                                and cum.get((eng, w.id), 0) >= w.wait_value):
                            continue
                        kept.append(w)
                    waits = kept
                if len(waits) > limit:
                    excess = waits[:-limit]
                    waits = waits[-limit:]
                    while excess:
                        take, excess = excess[:evsem_limit], excess[evsem_limit:]
                        ev = mybir.InstEventSemaphore(
                            name=nc.get_next_instruction_name(), ins=[], outs=[])
                        ev.engine = eng
                        ev.sync_info = bass_rust.SyncInfo(on_wait=take, on_update=[])
                        out.append(ev)
                    inst.sync_info = bass_rust.SyncInfo(on_wait=waits,
                                                        on_update=updates)
                    changed = True
                    n_fixed += 1
                elif si is not None and len(list(si.on_wait or [])) != len(waits):
                    inst.sync_info = bass_rust.SyncInfo(on_wait=waits,
                                                        on_update=updates)
                    changed = True
                    n_fixed += 1
                out.append(inst)
                if not _is_async_update(inst):
                    for u in updates:
                        if (u.sync_type == 'semaphore'
                                and u.update_mode == 'sem-inc'
                                and u.update_reg is None):
                            k = (eng, u.id)
                            cum[k] = cum.get(k, 0) + u.update_value
            if changed:
                bb.instructions = out
    return n_fixed


# ---------------------------------------------------------------------------
# Device kernel (per core: one batch image)
# ---------------------------------------------------------------------------

def build_conv_nc():
    nc = bass.Bass("TRN2", target_bir_lowering=False, debug=False)
    xin = nc.dram_tensor("xin", [4, 128, 64, 64], BF16, kind="ExternalInput").ap()
    wq = nc.dram_tensor("wq", [128, 4, 9, 2, 128], BF16, kind="ExternalInput").ap()
    noise4 = nc.dram_tensor("noise4", [4, 64, 64], BF16, kind="ExternalInput").ap()
    biasq = nc.dram_tensor("biasq", [128, 2], F32, kind="ExternalInput").ap()
    y4 = nc.dram_tensor("y4", [4, 2, 128, 64, 64], BF16, kind="ExternalOutput").ap()

    with ExitStack() as ctx:
        tc = ctx.enter_context(tile.TileContext(nc))
        xp = ctx.enter_context(tc.tile_pool(name="x", bufs=1))
        wp = ctx.enter_context(tc.tile_pool(name="w", bufs=1))
        np_ = ctx.enter_context(tc.tile_pool(name="noise", bufs=2))
        bp = ctx.enter_context(tc.tile_pool(name="bias", bufs=1))
        vp = ctx.enter_context(tc.tile_pool(name="v", bufs=2))
        up = ctx.enter_context(tc.tile_pool(name="u", bufs=1))
        tp = ctx.enter_context(tc.tile_pool(name="t", bufs=2))
        yp = ctx.enter_context(tc.tile_pool(name="y", bufs=1))
        op = ctx.enter_context(tc.tile_pool(name="o", bufs=2))
        pp = ctx.enter_context(tc.tile_pool(name="psum", bufs=8, space="PSUM"))

        # x with 1-pixel zero border: [128, cg, 66, 66]; w slabs.
        # DMA order: first x quarter of every cg first (matmul band 0
        # needs rows 0..15 of ALL cgs), then weights, then the rest.
        xq = xp.tile([128, 4, 66, 66], BF16)
        for cg in range(4):
            nc.gpsimd.memset(xq[:, cg, 0, :], 0.0)
            nc.gpsimd.memset(xq[:, cg, 65, :], 0.0)
            nc.gpsimd.memset(xq[:, cg, 1:65, 0], 0.0)
            nc.gpsimd.memset(xq[:, cg, 1:65, 65], 0.0)
        wslab = wp.tile([128, 4, 9, 2, 128], BF16)
        for cg in range(4):
            nc.sync.dma_start(xq[:, cg, 1:17, 1:65], xin[cg, :, 0:16, :])
        for cg in range(4):
            nc.sync.dma_start(wslab[:, cg], wq[:, cg])
        for qtr in range(1, 4):
            r = 16 * qtr
            for cg in range(4):
                nc.sync.dma_start(xq[:, cg, 1 + r:1 + r + 16, 1:65],
                                  xin[cg, :, r:r + 16, :])
        bias_sb = bp.tile([128, 2], F32)
        nc.sync.dma_start(bias_sb[:], biasq)
        alpha_sb = bp.tile([128, 1], F32)
        nc.gpsimd.memset(alpha_sb[:], 0.2)

        # full-resident u: [ci, p', b, ct, row, col]
        ut = up.tile([128, 2, 2, 2, 66, 64], BF16)

        # per-yband noise slabs, broadcast to all partitions: [128, ph, 16, 64]
        noise_t = {}

        def emit_noise(m):
            nt_ = np_.tile([128, 4, YBND, 64], BF16, tag="nt")
            nsrc = bass.AP(tensor=noise4.tensor, offset=YBND * m * 64,
                           ap=[[0, 128], [4096, 4], [64, YBND], [1, 64]])
            nc.sync.dma_start(nt_[:], nsrc)
            noise_t[m] = nt_

        emit_noise(0)
        emit_noise(1)

        # deferred per-yband work
        yt_tiles = {}

        def emit_ytaps(m):
            i0 = YBND * m
            yt = yp.tile([128, 4, 2, YBND, 64], BF16, tag="yt")
            for a in range(2):
                for b in range(2):
                    phi = 2 * a + b
                    u0 = ut[:, 0, b]
                    u1 = ut[:, 1, b]
                    s1 = tp.tile([128, 2, YBND, 64], BF16, tag="s1", bufs=2)
                    s2 = tp.tile([128, 2, YBND, 64], BF16, tag="s2", bufs=1)
                    if a == 0:
                        # y = (u0[i] + u1[i+1]) + 3*(u1[i] + u0[i+1])
                        nc.vector.tensor_tensor(
                            s1[:], u0[:, :, i0:i0 + YBND, :],
                            u1[:, :, i0 + 1:i0 + 1 + YBND, :], ADD)
                        nc.vector.tensor_tensor(
                            s2[:], u1[:, :, i0:i0 + YBND, :],
                            u0[:, :, i0 + 1:i0 + 1 + YBND, :], ADD)
                    else:
                        # y = (u1[i] + u0[i+2]) + 3*(u0[i+1] + u1[i+1])
                        nc.vector.tensor_tensor(
                            s1[:], u1[:, :, i0:i0 + YBND, :],
                            u0[:, :, i0 + 2:i0 + 2 + YBND, :], ADD)
                        nc.vector.tensor_tensor(
                            s2[:], u0[:, :, i0 + 1:i0 + 1 + YBND, :],
                            u1[:, :, i0 + 1:i0 + 1 + YBND, :], ADD)
                    nc.vector.tensor_scalar(s2[:], s2[:], 3.0, None, MULT)
                    nc.vector.tensor_tensor(s1[:], s1[:], s2[:], ADD)
                    # + noise (per ct: noise has no ct dim)
                    for ct in range(2):
                        nc.vector.tensor_tensor(
                            yt[:, phi, ct], s1[:, ct],
                            noise_t[m][:, phi], ADD)
            yt_tiles[m] = yt

        def emit_yacts(m):
            i0 = YBND * m
            yt = yt_tiles.pop(m)
            ot = op.tile([128, 4, 2, YBND, 64], BF16, tag="ot")
            for phi in range(4):
                for ct in range(2):
                    nc.scalar.activation(
                        ot[:, phi, ct], yt[:, phi, ct],
                        mybir.ActivationFunctionType.Prelu,
                        bias=bias_sb[:, ct:ct + 1], scale=1.0,
                        alpha=alpha_sb[:, 0:1])
            ydst = bass.AP(
                tensor=y4.tensor, offset=i0 * 64,
                ap=[[4096, 128], [1048576, 4], [524288, 2], [64, YBND], [1, 64]])
            nc.sync.dma_start(ydst, ot[:])

        taps_done = 0
        acts_done = 0
        for k in range(NB):
            r0 = BND * k
            vt = vp.tile([128, 2, 4, BND, 66], BF16, tag="vt")
            for phi, ((pp_, qq), taps) in enumerate(PHASES):
                nr = min(BND, ROWS[pp_] - r0)
                cols = COLS[qq]
                halves = [(0, min(7, nr))]
                if nr > 7:
                    halves.append((7, nr - 7))
                for ct in range(2):
                    pst = [pp.tile([128, 7, 66], F32, tag="ps", name=f"ps{hh}")
                           for hh, _ in enumerate(halves)]
                    nt = len(taps)
                    for cg in range(4):
                        for it, (t, dy, dx) in enumerate(taps):
                            first = (cg == 0 and it == 0)
                            last = (cg == 3 and it == nt - 1)
                            for (h, hn), ps in zip(halves, pst):
                                nc.tensor.matmul(
                                    ps[:, 0:hn, 0:cols],
                                    wslab[:, cg, t, ct, :],
                                    xq[:, cg,
                                       r0 + h + dy:r0 + h + dy + hn,
                                       dx:dx + cols],
                                    start=first, stop=last,
                                    skip_group_check=True)
                    for (h, hn), ps in zip(halves, pst):
                        nc.scalar.copy(vt[:, ct, phi, h:h + hn, 0:cols],
                                       ps[:, 0:hn, 0:cols])
            # column FIR: u[p', b] rows of this band (no row halo needed)
            for p in range(2):
                nr = min(BND, ROWS[p] - r0)
                if nr <= 0:
                    continue
                v0 = vt[:, :, 2 * p + 0]
                v1 = vt[:, :, 2 * p + 1]
                for b in range(2):
                    usl = ut[:, p, b, :, r0:r0 + nr, :]
                    s2 = tp.tile([128, 2, BND, 64], BF16, tag="us2", bufs=1)
                    if b == 0:
                        # u = (v0[j] + v1[j+1]) + 3*(v1[j] + v0[j+1])
                        nc.vector.tensor_tensor(usl,
                                                v0[:, :, 0:nr, 0:64],
                                                v1[:, :, 0:nr, 1:65], ADD)
                        nc.vector.tensor_tensor(s2[:, :, 0:nr, :],
                                                v1[:, :, 0:nr, 0:64],
                                                v0[:, :, 0:nr, 1:65], ADD)
                    else:
                        # u = (v1[j] + v0[j+2]) + 3*(v0[j+1] + v1[j+1])
                        nc.vector.tensor_tensor(usl,
                                                v1[:, :, 0:nr, 0:64],
                                                v0[:, :, 0:nr, 2:66], ADD)
                        nc.vector.tensor_tensor(s2[:, :, 0:nr, :],
                                                v0[:, :, 0:nr, 1:65],
                                                v1[:, :, 0:nr, 1:65], ADD)
                    nc.vector.tensor_scalar(s2[:, :, 0:nr, :],
                                            s2[:, :, 0:nr, :], 3.0, None, MULT)
                    nc.vector.tensor_tensor(usl, usl,
                                            s2[:, :, 0:nr, :], ADD)
            # deferred epilogue for the previous yband, then new ytaps
            while acts_done < taps_done:
                emit_yacts(acts_done)
                acts_done += 1
            while taps_done < NYB and YGATE[taps_done] <= k:
                emit_ytaps(taps_done)
                taps_done += 1
                if taps_done + 1 < NYB and (taps_done + 1) not in noise_t:
                    emit_noise(taps_done + 1)
        while acts_done < NYB:
            if taps_done < NYB:
                emit_ytaps(taps_done)
                taps_done += 1
            emit_yacts(acts_done)
            acts_done += 1

    legalize_waits(nc)
    return nc


# ---------------------------------------------------------------------------
# Host-side preparation (renorm + flip + phase decomposition, all in fp64)
# ---------------------------------------------------------------------------

def prep_inputs(x, weight, bias, noise_const, noise_strength):
    from ml_dtypes import bfloat16
    SQ2 = np.sqrt(2.0)
    w = np.asarray(weight, np.float64)
    inv = 1.0 / np.sqrt((w ** 2).sum(axis=(1, 2, 3)) + 1e-8)
    wf = (w * inv[:, None, None, None])[:, :, ::-1, ::-1] * (SQ2 / 16.0)
    W = wf.transpose(1, 0, 2, 3)                 # [ci, co, u, v]

    wq = np.empty((128, 4, 9, 2, 128), np.float32)
    for t, (u, v) in enumerate(TAPLIST):
        for cg in range(4):
            for ct in range(2):
                wq[:, cg, t, ct, :] = W[cg * 128:(cg + 1) * 128,
                                        ct * 128:(ct + 1) * 128, u, v]
    wq = np.ascontiguousarray(wq.astype(bfloat16))

    s = float(np.asarray(noise_strength)) * SQ2
    nco = np.asarray(noise_const, np.float64)
    noise4 = np.empty((4, 64, 64), np.float32)
    for a in range(2):
        for b in range(2):
            noise4[2 * a + b] = nco[a::2, b::2] * s
    noise4 = np.ascontiguousarray(noise4.astype(bfloat16))

    biasq = np.empty((128, 2), np.float32)
    b64 = np.asarray(bias, np.float64) * SQ2
    biasq[:, 0] = b64[:128]
    biasq[:, 1] = b64[128:]

    xb = np.asarray(x, np.float32).astype(bfloat16)   # [8, 512, 64, 64]
    return [{
        "xin": np.ascontiguousarray(xb[i].reshape(4, 128, 64, 64)),
        "wq": wq,
        "noise4": noise4,
        "biasq": biasq,
    } for i in range(x.shape[0])]


_NC_CACHE = None


def kernel(x, weight, bias, noise_const, noise_strength):
    global _NC_CACHE
    if _NC_CACHE is None:
        _NC_CACHE = build_conv_nc()
    in_maps = prep_inputs(x, weight, bias, noise_const, noise_strength)
    res = run_bass_kernel_spmd(_NC_CACHE, in_maps, core_ids=list(range(8)))
    out = np.empty((x.shape[0], 256, 128, 128), np.float32)
    for i, r in enumerate(res.results):
        yq = np.asarray(r["y4"]).astype(np.float32)   # [4, 2, 128, 64, 64]
        for a in range(2):
            for b in range(2):
                out[i, :, a::2, b::2] = yq[2 * a + b].reshape(256, 64, 64)
    return out
